# revision 19
# baseline (speedup 1.0000x reference)
"""MoE (top-1 routed) Trainium2 kernel.

Strategy: the reference computes every expert for every token and then
selects one expert per token with a one-hot gate.  Mathematically the
output for token n is expert_out[argmax_e logits[n, e], n], so we compute
the gating on host (bitwise-matching the reference's fp32 `x @ Wg + bg`
on CPU), group tokens by their selected expert, and run expert e's
pipeline for only its own tokens on NeuronCore e (expert-parallel, an
all-reduce-free gather).  This is 8x less device compute than the dense
reference formulation.

Device pipeline per core (C = padded token count, transposed layout with
features on partitions and tokens on the free dim):
    h^T[u, n]  = W1^T x^T          (PE, K=1024 accumulated in PSUM)
    sw         = (tanh(h/2) + 1) * h            # == 2*swish(h)
    z^T[v, n]  = (0.5*proj)^T sw   (PE)         # 0.5 folds the 2 above
    t2         = tanh(z/2)                      # == 2*sigmoid(z) - 1
    g_j        = exp(32*k_j*t2 + 32*k_j*(1-k_j))   j=1..7   (g_0 == 1)
      -- g_j is the reference's gaussian basis exp(-32*(xn-k_j)^2) times
         exp(32*xn^2), a per-element factor that cancels in the
         normalization below (the reference's +1e-6 in the denominator is
         a <=1.2e-6 relative perturbation, below fp32 matmul noise).
    den        = 1 + sum_j g_j                  (GPSIMD add tree)
    num        = cv_0 + sum_j g_j * cv_j        # cv = ctrl * scaling
                                                (DVE fused mul-add chain)
    out^T[u,n] = num * reciprocal(den)

tanh and exp share one ACT table set ("exp_and_others"), so the scalar
engine never pays the ~2.7us table switch.  swish(x) = x*sigmoid(x)
= 0.5*x*(1+tanh(x/2)) and sigmoid(z) = 0.5*(1+tanh(z/2)) are exact
identities, with constants folded into proj / the exp arguments.

Matmul dtype modes: "f32" (exact, 4 PE cycles/row), "f32r" (full-rate
fp32 PE path, ~1.5e-4 relative error, measured on hw), "bf16".
"""

import os
from contextlib import ExitStack

import numpy as np

N_TOK, D_IN, U_DIM, E_EXP, B_BAS = 8192, 1024, 512, 8, 8
N_CORES = 8
P = 128
TNMAX = 512

MM_MODE = os.environ.get("MOE_MM_MODE", "f32r")

_prog_cache = {}


def _knot_consts():
    ks = np.linspace(0.0, 1.0, B_BAS).astype(np.float64)
    scales = 32.0 * ks
    biases = 32.0 * ks * (1.0 - ks)
    return ks, scales, biases


def build_program(C, mm_mode, b1_zero):
    """Build + compile the SPMD single-core program for capacity C."""
    import concourse.tile as tile
    from concourse import bacc, mybir

    f32 = mybir.dt.float32
    add = mybir.AluOpType.add
    mult = mybir.AluOpType.mult
    Tanh = mybir.ActivationFunctionType.Tanh
    Exp = mybir.ActivationFunctionType.Exp

    if mm_mode == "bf16":
        mm_dt = mybir.dt.bfloat16
    elif mm_mode == "f32r":
        mm_dt = mybir.dt.float32r
    else:
        mm_dt = f32

    assert C % P == 0
    tiles = []
    t0 = 0
    while C - t0 >= TNMAX:
        tiles.append((t0, TNMAX))
        t0 += TNMAX
    if C - t0 > 0:
        tiles.append((t0, C - t0))

    _, escale, ebias = _knot_consts()

    nc = bacc.Bacc("TRN2", target_bir_lowering=False, debug=False,
                   num_devices=N_CORES)

    xT = nc.dram_tensor("xT", [D_IN, C], mm_dt, kind="ExternalInput").ap()
    w1 = nc.dram_tensor("w1", [D_IN, U_DIM], mm_dt, kind="ExternalInput").ap()
    p5 = nc.dram_tensor("p5", [U_DIM, U_DIM], mm_dt, kind="ExternalInput").ap()
    cv = nc.dram_tensor("cv", [P, 4, B_BAS], f32, kind="ExternalInput").ap()
    aux = nc.dram_tensor("aux", [33, P, P], mybir.dt.float32r,
                         kind="ExternalInput").ap()
    onesd = nc.dram_tensor("onesd", [P, TNMAX], mybir.dt.float32r,
                           kind="ExternalInput").ap()
    b1h = nc.dram_tensor("b1h", [P, 4], f32, kind="ExternalInput").ap()
    outT = nc.dram_tensor("outT", [U_DIM, C], f32, kind="ExternalOutput").ap()

    xT_r = xT.rearrange("(kc p) c -> p kc c", p=P)
    aux_r = aux.rearrange("a p q -> p a q")
    w1_r = w1.rearrange("(kc p) u -> p kc u", p=P)
    p5_r = p5.rearrange("(uc p) v -> p uc v", p=P)
    outT_r = outT.rearrange("(vc p) c -> p vc c", p=P)

    with tile.TileContext(nc) as tc, ExitStack() as ctx:
        cpool = ctx.enter_context(tc.tile_pool(name="consts", bufs=1))
        xpool = ctx.enter_context(tc.tile_pool(name="x", bufs=3))
        pspool = ctx.enter_context(tc.tile_pool(name="ps", bufs=8, space="PSUM"))
        epool = ctx.enter_context(tc.tile_pool(name="elem", bufs=3))
        swpool = ctx.enter_context(tc.tile_pool(name="sw", bufs=6))
        gpool = ctx.enter_context(tc.tile_pool(name="g", bufs=8))
        mpool = ctx.enter_context(tc.tile_pool(name="m", bufs=4))
        tpool = ctx.enter_context(tc.tile_pool(name="t", bufs=2))
        opool = ctx.enter_context(tc.tile_pool(name="o", bufs=2))

        use_pe_basis = (mm_mode == "f32r")
        PE_VCS = (0, 1) if use_pe_basis else ()

        # x token tiles: issue ALL loads first so tile 0's data races the
        # (larger) weight loads instead of queueing behind them
        xq = []
        for (t0, TN) in tiles:
            xa = xpool.tile([P, 4, TNMAX], mm_dt, tag="xa",
                            name=f"xa{t0}")
            nc.sync.dma_start(xa[:, :, :TN], xT_r[:, 0:4, t0:t0 + TN])
            xb = xpool.tile([P, 4, TNMAX], mm_dt, tag="xb",
                            name=f"xb{t0}")
            nc.sync.dma_start(xb[:, :, :TN], xT_r[:, 4:8, t0:t0 + TN])
            xq.append((xa, xb))

        # resident weights on the ACT queue (parallel with x on sync)
        w1k = []
        for kc in range(8):
            t = cpool.tile([P, U_DIM], mm_dt, tag=f"w1_{kc}")
            nc.scalar.dma_start(t[:], w1_r[:, kc, :])
            w1k.append(t)
        puc = []
        for uc in range(4):
            t = cpool.tile([P, U_DIM], mm_dt, tag=f"p5_{uc}")
            eng = nc.sync if uc % 2 == 0 else nc.scalar
            eng.dma_start(t[:], p5_r[:, uc, :])
            puc.append(t)
        # small/late-needed constants via the gpsimd SWDGE queue
        cvsb = cpool.tile([P, 4, B_BAS], f32, tag="cv")
        nc.gpsimd.dma_start(cvsb[:], cv[:])
        ebsb = cpool.tile([P, 8], f32, tag="ebias")
        for j in range(1, 8):
            nc.gpsimd.memset(ebsb[:, j:j + 1], float(ebias[j]))
        ones = cpool.tile([P, TNMAX], mm_dt if use_pe_basis else f32,
                          tag="ones")
        if use_pe_basis:
            nc.gpsimd.dma_start(ones[:], onesd[:])
        else:
            nc.gpsimd.memset(ones[:], 1.0)
        if use_pe_basis:
            auxsb = cpool.tile([P, 33, P], mm_dt, tag="aux")
            nc.gpsimd.dma_start(auxsb[:], aux_r[:])
        if not b1_zero:
            b1sb = cpool.tile([P, 4], f32, tag="b1h")
            nc.gpsimd.dma_start(b1sb[:], b1h[:])

        for ti, (t0, TN) in enumerate(tiles):
            xa, xb = xq[ti]

            sws = []
            for uc in range(4):
                hps = pspool.tile([P, TNMAX], f32, tag="ps", name="hps")
                for kc in range(8):
                    xt = xa if kc < 4 else xb
                    nc.tensor.matmul(
                        hps[:, :TN],
                        lhsT=w1k[kc][:, uc * P:(uc + 1) * P],
                        rhs=xt[:, kc % 4, :TN],
                        start=(kc == 0), stop=(kc == 7),
                    )
                th = epool.tile([P, TNMAX], f32, tag="th")
                if b1_zero:
                    nc.scalar.activation(th[:, :TN], hps[:, :TN], Tanh, scale=0.5)
                else:
                    nc.scalar.activation(th[:, :TN], hps[:, :TN], Tanh,
                                         scale=0.5, bias=b1sb[:, uc:uc + 1])
                sw = swpool.tile([P, TNMAX], mm_dt, tag="sw")
                if b1_zero:
                    # sw = (th + 1) * h  == 2*swish(h)
                    nc.vector.scalar_tensor_tensor(
                        sw[:, :TN], th[:, :TN], 1.0, hps[:, :TN], op0=add, op1=mult)
                else:
                    y = epool.tile([P, TNMAX], f32, tag="y")
                    nc.vector.tensor_scalar(
                        y[:, :TN], hps[:, :TN], b1sb[:, uc:uc + 1], None, op0=add)
                    nc.vector.scalar_tensor_tensor(
                        sw[:, :TN], th[:, :TN], 1.0, y[:, :TN], op0=add, op1=mult)
                sws.append(sw)

            gdt = mm_dt if use_pe_basis else f32
            outb = opool.tile([P, 4, TNMAX], f32, tag="outb")
            gs = [None] * 4
            for vc in range(4):
                zps = pspool.tile([P, TNMAX], f32, tag="ps", name="zps")
                for uc in range(4):
                    nc.tensor.matmul(
                        zps[:, :TN],
                        lhsT=puc[uc][:, vc * P:(vc + 1) * P],
                        rhs=sws[uc][:, :TN],
                        start=(uc == 0), stop=(uc == 3),
                    )
                t2 = epool.tile([P, TNMAX], f32, tag="t2")
                nc.scalar.activation(t2[:, :TN], zps[:, :TN], Tanh, scale=0.5)

                g = [None] * 8
                for j in range(1, 8):
                    g[j] = gpool.tile([P, TNMAX], gdt, tag="g", name=f"g{j}")
                    nc.scalar.activation(g[j][:, :TN], t2[:, :TN], Exp,
                                         scale=float(escale[j]),
                                         bias=ebsb[:, j:j + 1])
                gs[vc] = g

                if vc in PE_VCS:
                    # num and den as accumulating diag/identity matmuls on PE
                    nps = pspool.tile([P, TNMAX], f32, tag="ps", name="nps")
                    dps = pspool.tile([P, TNMAX], f32, tag="ps", name="dps")
                    nc.tensor.matmul(nps[:, :TN], lhsT=auxsb[:, vc * 8, :],
                                     rhs=ones[:, :TN], start=True, stop=False)
                    for j in range(1, 8):
                        nc.tensor.matmul(nps[:, :TN],
                                         lhsT=auxsb[:, vc * 8 + j, :],
                                         rhs=g[j][:, :TN],
                                         start=False, stop=(j == 7))
                    nc.tensor.matmul(dps[:, :TN], lhsT=auxsb[:, 32, :],
                                     rhs=ones[:, :TN], start=True, stop=False)
                    for j in range(1, 8):
                        nc.tensor.matmul(dps[:, :TN], lhsT=auxsb[:, 32, :],
                                         rhs=g[j][:, :TN],
                                         start=False, stop=(j == 7))
                    r = mpool.tile([P, TNMAX], f32, tag="r", name=f"r{vc}")
                    nc.vector.reciprocal_approx_fast(r[:, :TN], dps[:, :TN])
                    nc.vector.tensor_tensor(
                        outb[:, vc, :TN], nps[:, :TN], r[:, :TN], mult)

            # DVE-path vc chunks: interleave the two chains so neither
            # engine's FIFO head-of-line blocks on a serial dependency
            dve_vcs = [vc for vc in range(4) if vc not in PE_VCS]
            gf = {}
            onesf = ones.bitcast(f32) if ones.dtype != f32 else ones
            for vc in dve_vcs:
                g = gs[vc]
                gf[vc] = [None] + [
                    (g[j].bitcast(f32) if g[j].dtype != f32 else g[j])
                    for j in range(1, 8)]
            et = {}
            for (a, b, ta) in [(1, 2, "p12"), (3, 4, "p34"), (5, 6, "p56")]:
                for vc in dve_vcs:
                    e_a = tpool.tile([P, TNMAX], f32, tag=ta, name=f"{ta}v{vc}")
                    nc.gpsimd.tensor_tensor(
                        e_a[:, :TN], gf[vc][a][:, :TN], gf[vc][b][:, :TN], add)
                    et[(vc, ta)] = e_a
            for vc in dve_vcs:
                e_b = tpool.tile([P, TNMAX], f32, tag="p78", name=f"p78v{vc}")
                nc.gpsimd.tensor_tensor(
                    e_b[:, :TN], gf[vc][7][:, :TN], onesf[:, :TN], add)
                et[(vc, "p78")] = e_b
            for vc in dve_vcs:
                e_b = tpool.tile([P, TNMAX], f32, tag="q14", name=f"q14v{vc}")
                nc.gpsimd.tensor_tensor(
                    e_b[:, :TN], et[(vc, "p12")][:, :TN], et[(vc, "p34")][:, :TN], add)
                et[(vc, "q14")] = e_b
            for vc in dve_vcs:
                e_b = tpool.tile([P, TNMAX], f32, tag="q58", name=f"q58v{vc}")
                nc.gpsimd.tensor_tensor(
                    e_b[:, :TN], et[(vc, "p56")][:, :TN], et[(vc, "p78")][:, :TN], add)
                et[(vc, "q58")] = e_b
            den = {}
            for vc in dve_vcs:
                d_ = tpool.tile([P, TNMAX], f32, tag="dd", name=f"denv{vc}")
                nc.gpsimd.tensor_tensor(
                    d_[:, :TN], et[(vc, "q14")][:, :TN], et[(vc, "q58")][:, :TN], add)
                den[vc] = d_
            # num chains, interleaved
            mcur = {}
            for vc in dve_vcs:
                m = mpool.tile([P, TNMAX], f32, tag="num", name=f"m1v{vc}")
                nc.vector.scalar_tensor_tensor(
                    m[:, :TN], gf[vc][1][:, :TN], cvsb[:, vc, 1:2],
                    cvsb[:, vc, 0:1].to_broadcast([P, TN]), op0=mult, op1=add)
                mcur[vc] = m
            for j in range(2, 8):
                for vc in dve_vcs:
                    m2 = mpool.tile([P, TNMAX], f32, tag="num", name=f"m{j}v{vc}")
                    nc.vector.scalar_tensor_tensor(
                        m2[:, :TN], gf[vc][j][:, :TN], cvsb[:, vc, j:j + 1],
                        mcur[vc][:, :TN], op0=mult, op1=add)
                    mcur[vc] = m2
            for vc in dve_vcs:
                r = mpool.tile([P, TNMAX], f32, tag="r", name=f"r{vc}")
                nc.vector.reciprocal_approx_fast(r[:, :TN], den[vc][:, :TN])
                nc.vector.tensor_tensor(
                    outb[:, vc, :TN], mcur[vc][:, :TN], r[:, :TN], mult)

            nc.sync.dma_start(outT_r[:, :, t0:t0 + TN], outb[:, :, :TN])

    nc.compile()
    return nc, tiles


def _get_program(C, mm_mode, b1_zero):
    key = (C, mm_mode, b1_zero)
    if key not in _prog_cache:
        _prog_cache[key] = build_program(C, mm_mode, b1_zero)
    return _prog_cache[key]


def _route_on_host(x, Wg, bg):
    """Expert assignment, bitwise-matching the reference's fp32 CPU math."""
    import jax
    import jax.numpy as jnp

    cpu = jax.devices("cpu")[0]
    with jax.default_device(cpu):
        logits = jnp.asarray(x) @ jnp.asarray(Wg) + jnp.asarray(bg)
        eid = np.asarray(jnp.argmax(logits, axis=-1))
    return eid


def make_in_maps(x, W1, b1, proj, ctrl, scaling, Wg, bg, mm_mode):
    import ml_dtypes

    x = np.asarray(x, dtype=np.float32)
    eid = _route_on_host(x, Wg, bg)
    order = np.argsort(eid, kind="stable")
    counts = np.bincount(eid, minlength=E_EXP)
    starts = np.zeros(E_EXP + 1, dtype=np.int64)
    starts[1:] = np.cumsum(counts)
    C = int(max(counts.max(), 1))
    C = ((C + P - 1) // P) * P

    mm_np = ml_dtypes.bfloat16 if mm_mode == "bf16" else np.float32

    cvf = (np.asarray(ctrl, np.float32)
           * np.asarray(scaling, np.float32)[:, None, :])  # [E, B, U]
    proj5 = 0.5 * np.asarray(proj, np.float32)
    b1f = np.asarray(b1, np.float32)
    b1_zero = not np.any(b1f)

    in_maps = []
    for e in range(E_EXP):
        idx = order[starts[e]:starts[e + 1]]
        xT = np.zeros((D_IN, C), dtype=mm_np)
        if len(idx):
            xT[:, :len(idx)] = x[idx].T
        # cv_dev[p, vc, j] = cv[e, j, vc*128+p]
        cv_dev = np.ascontiguousarray(
            cvf[e].T.reshape(4, P, B_BAS).transpose(1, 0, 2)).astype(np.float32)
        b1h = np.ascontiguousarray(
            (0.5 * b1f[e]).reshape(4, P).T).astype(np.float32)
        # aux[vc*8+j] = diag(cv[e, j, vc*128:(vc+1)*128]); aux[32] = I
        aux = np.zeros((33, P, P), dtype=np.float32)
        ar = np.arange(P)
        for vc in range(4):
            for j in range(B_BAS):
                aux[vc * 8 + j, ar, ar] = cvf[e][j, vc * P:(vc + 1) * P]
        aux[32, ar, ar] = 1.0
        in_maps.append({
            "xT": xT,
            "w1": np.asarray(W1[e], np.float32).astype(mm_np),
            "p5": proj5[e].astype(mm_np),
            "cv": cv_dev,
            "b1h": b1h,
            "aux": aux,
            "onesd": np.ones((P, TNMAX), dtype=np.float32),
        })
    return in_maps, order, starts, counts, C, b1_zero


def kernel(x, W1, b1, proj, ctrl, scaling, Wg, bg):
    from concourse.bass_utils import run_bass_kernel_spmd

    mm_mode = MM_MODE
    in_maps, order, starts, counts, C, b1_zero = make_in_maps(
        x, W1, b1, proj, ctrl, scaling, Wg, bg, mm_mode)
    nc, _ = _get_program(C, mm_mode, b1_zero)

    res = run_bass_kernel_spmd(nc, in_maps, list(range(N_CORES)))

    out = np.empty((N_TOK, U_DIM), dtype=np.float32)
    for e in range(E_EXP):
        cnt = int(counts[e])
        if cnt:
            out[order[starts[e]:starts[e + 1]]] = res.results[e]["outT"][:, :cnt].T
    return out
